# revision 13
# baseline (speedup 1.0000x reference)
"""MultiHeadAttention Trainium2 Bass kernel (8-core SPMD).

Problem: B=2, S=2048, DIM=1024, H=16 heads (dh=64), fp32 reference.
Sharding: core c handles batch b = c//4 and 4 heads ho = 4*(c%4)..+4
(data-parallel over batch x tensor-parallel over heads). Each core:
  qhT/khT = W{q,k}.T-slice @ x.T + b   -> [256, 2048] bf16 (head-dim major)
  vh_aug  = x @ Wv.T-slice + bv (+ones col per head) per k-tile
  scores^T = kh @ qh.T (per head, K=64 row-packed pairs)
  P^T = keepmask * exp(SCALE * scores^T)                (ACT + DVE)
  out^T[65|128, q] = [vh | ones].T @ P^T                (PV + row-sums fused)
  normalize by 1/sums (batched reciprocal), y^T_partial = Wo.T-slice.T @ O^T
Host gathers: y[b] = sum over 4 cores of y^T_partial.T, + bo.
"""

import os
import sys

sys.path.insert(0, "/opt/trn_rl_repo")
os.environ.setdefault("MYCRO_LOCAL_CACHE", "1")

import numpy as np

import concourse.bass as bass
import concourse.bacc as bacc
import concourse.tile as tile
from concourse import mybir
from concourse import bass_utils

F32 = mybir.dt.float32
BF16 = mybir.dt.bfloat16
NP_BF16 = mybir.dt.np(BF16)

B, S, DIM = 2, 2048, 1024
H = 16
DH = 64
SCALE = 1.0 / (DIM ** 0.5)
N_CORES = 8
HPC = 4          # heads per core
QT = S // 512    # 4 q-chunks of 512
KT = S // 128    # 16 k-tiles of 128
CT = DIM // 128  # 8 contraction tiles for projections

# vh_aug per-kt layout: per pair p (2 local pairs):
#   A block: [vh_A(64) | ones(1)]                 at cols p*193 + [0, 65)
#   B block: [zeros(32) | ones(1) | zeros(31) | vh_B(64)] at cols p*193 + [65, 193)
#   (B ones at col +97 so B sums land on psum partition 32 — DVE start
#   partitions must be in {0, 32, 64, 96})
VHA_W = 386


def build_nc():
    # Bacc (not plain Bass): its compile() pipeline splits multi-semaphore
    # waits into event-semaphore chains — walrus codegen allows only ONE
    # sync wait per compute instruction on TRN2.
    nc = bacc.Bacc("TRN2", target_bir_lowering=False)

    xqT_d = nc.declare_dram_parameter("xqT", [DIM, S], BF16, isOutput=False)
    xkT_d = nc.declare_dram_parameter("xkT", [DIM, S], BF16, isOutput=False)
    xvT_d = nc.declare_dram_parameter("xvT", [DIM, S], BF16, isOutput=False)
    wqT_d = nc.declare_dram_parameter("wqT", [DIM, 256], BF16, isOutput=False)
    wkT_d = nc.declare_dram_parameter("wkT", [DIM, 256], BF16, isOutput=False)
    wvT_d = nc.declare_dram_parameter("wvT", [DIM, 256], BF16, isOutput=False)
    woT_d = nc.declare_dram_parameter("woT", [256, DIM], BF16, isOutput=False)
    bq_d = nc.declare_dram_parameter("bq2", [2, 128, 1], F32, isOutput=False)
    bk_d = nc.declare_dram_parameter("bk2", [2, 128, 1], F32, isOutput=False)
    bvb_d = nc.declare_dram_parameter("bvb", [128, 256], BF16, isOutput=False)
    maskT_d = nc.declare_dram_parameter("maskT", [S, S], BF16, isOutput=False)
    yT_d = nc.declare_dram_parameter("yT", [DIM, S], F32, isOutput=True)
    # internal DRAM bounce for the per-(head,q) reciprocal rows: SBUF APs
    # cannot broadcast along partitions (nonzero-step required), DRAM APs can
    rscr_d = nc.dram_tensor("rscr", [HPC, S], F32)

    with tile.TileContext(nc) as tc:
        with tc.tile_pool(name="persist", bufs=1) as singles:
            # ---- load inputs to SBUF ----
            def load_rows(dram, n_tiles, width, tag):
                tiles = []
                for c in range(n_tiles):
                    t = singles.tile([128, width], BF16, tag=f"{tag}{c}", name=f"{tag}{c}")
                    nc.sync.dma_start(out=t, in_=dram[c * 128:(c + 1) * 128, :])
                    tiles.append(t)
                return tiles

            xq_sb = load_rows(xqT_d, CT, S, "xq")
            xk_sb = load_rows(xkT_d, CT, S, "xk")
            xv_sb = load_rows(xvT_d, CT, S, "xv")
            wq_sb = load_rows(wqT_d, CT, 256, "wq")
            wk_sb = load_rows(wkT_d, CT, 256, "wk")
            wv_sb = load_rows(wvT_d, CT, 256, "wv")
            wo_sb = load_rows(woT_d, 2, DIM, "wo")

            bq_sb, bk_sb = [], []
            for m in range(2):
                tq = singles.tile([128, 1], F32, tag=f"bq{m}", name=f"bq{m}")
                nc.sync.dma_start(out=tq, in_=bq_d[m])
                bq_sb.append(tq)
                tk = singles.tile([128, 1], F32, tag=f"bk{m}", name=f"bk{m}")
                nc.sync.dma_start(out=tk, in_=bk_d[m])
                bk_sb.append(tk)
            bvb_sb = singles.tile([128, 256], BF16, tag="bvb")
            nc.sync.dma_start(out=bvb_sb, in_=bvb_d[:, :])

            # ---- persistent intermediates ----
            qhT = [singles.tile([128, S], BF16, tag=f"qhT{m}", name=f"qhT{m}") for m in range(2)]
            khT = [singles.tile([128, S], BF16, tag=f"khT{m}", name=f"khT{m}") for m in range(2)]
            OT = [singles.tile([128, S], BF16, tag=f"OT{m}", name=f"OT{m}") for m in range(2)]
            vha = singles.tile([128, KT, VHA_W], BF16, tag="vha")
            # sums staging: pair p's A-head sums at partition 64, B at 63,
            # free dim [pair, q] (DVE is lane-locked; DMA rearranges later)
            sums_stage = singles.tile([128, 2, S], F32, tag="sums_stage")

            # constants in vh_aug (ones + zero pad), on gpsimd
            for p in range(2):
                base = p * 193
                nc.gpsimd.memset(vha[:, :, base + 64:base + 65], 1.0)    # A ones
                nc.gpsimd.memset(vha[:, :, base + 97:base + 98], 1.0)    # B ones
                nc.gpsimd.memset(vha[:, :, base + 65:base + 97], 0.0)    # B zero pad
                nc.gpsimd.memset(vha[:, :, base + 98:base + 129], 0.0)   # B zero pad

            # ---- phase P: projections ----
            with tc.tile_pool(name="pjp", bufs=2, space="PSUM") as pj:
                for x_sb, w_sb, b_sb, dst in (
                    (xq_sb, wq_sb, bq_sb, qhT),
                    (xk_sb, wk_sb, bk_sb, khT),
                ):
                    for m in range(2):
                        for n in range(QT):
                            ps = pj.tile([128, 512], F32, tag="pqk")
                            for c in range(CT):
                                nc.tensor.matmul(
                                    ps,
                                    w_sb[c][:, m * 128:(m + 1) * 128],
                                    x_sb[c][:, n * 512:(n + 1) * 512],
                                    start=(c == 0),
                                    stop=(c == CT - 1),
                                )
                            bb = b_sb[m][:, 0:1]
                            bb_bc = bass.AP(
                                tensor=bb.tensor, offset=bb.offset,
                                ap=[list(bb.ap[0]), [0, 512]])
                            nc.vector.tensor_tensor(
                                out=dst[m][:, n * 512:(n + 1) * 512],
                                in0=ps,
                                in1=bb_bc,
                                op=mybir.AluOpType.add,
                            )
                for kt in range(KT):
                    ps = pj.tile([128, 256], F32, tag="pv")
                    for c in range(CT):
                        nc.tensor.matmul(
                            ps,
                            xv_sb[c][:, kt * 128:(kt + 1) * 128],
                            wv_sb[c],
                            start=(c == 0),
                            stop=(c == CT - 1),
                        )
                    for h in range(HPC):
                        p, is_b = h // 2, h % 2
                        col = p * 193 + (129 if is_b else 0)
                        nc.vector.tensor_tensor(
                            out=vha[:, kt, col:col + 64],
                            in0=ps[:, h * 64:(h + 1) * 64],
                            in1=bvb_sb[:, h * 64:(h + 1) * 64],
                            op=mybir.AluOpType.add,
                        )

            # ---- phase A: attention ----
            with tc.tile_pool(name="scp", bufs=2, space="PSUM") as scp, \
                 tc.tile_pool(name="pvp", bufs=2, space="PSUM") as pvp, \
                 tc.tile_pool(name="ptp", bufs=4) as ptp, \
                 tc.tile_pool(name="mkp", bufs=3) as mkp:
                for qt in range(QT):
                    po = [pvp.tile([128, 1024], F32, tag="po", name="po") for _ in range(2)]
                    for kt in range(KT):
                        mt = mkp.tile([128, 512], BF16, tag="mask")
                        nc.sync.dma_start(
                            out=mt,
                            in_=maskT_d[kt * 128:(kt + 1) * 128,
                                        qt * 512:(qt + 1) * 512],
                        )
                        m_ap = mt[:, :]
                        mbc = bass.AP(
                            tensor=m_ap.tensor,
                            offset=m_ap.offset,
                            ap=[list(m_ap.ap[0]), [0, 2], list(m_ap.ap[1])],
                        )
                        for p in range(2):
                            ps = scp.tile([128, 1024], F32, tag="sc")
                            for ab in range(2):
                                nc.tensor.matmul(
                                    ps[:, ab * 512:(ab + 1) * 512],
                                    khT[p][ab * 64:(ab + 1) * 64,
                                           kt * 128:(kt + 1) * 128],
                                    qhT[p][ab * 64:(ab + 1) * 64,
                                           qt * 512:(qt + 1) * 512],
                                    start=True,
                                    stop=True,
                                )
                            pt = ptp.tile([128, 1024], BF16, tag="pt")
                            nc.scalar.activation(
                                out=pt, in_=ps,
                                func=mybir.ActivationFunctionType.Exp,
                                scale=float(SCALE),
                            )
                            nc.vector.tensor_tensor(
                                out=pt, in0=pt, in1=mbc,
                                op=mybir.AluOpType.mult,
                            )
                            base = p * 193
                            nc.tensor.matmul(
                                po[p][0:65, 0:512],
                                vha[:, kt, base:base + 65],
                                pt[:, 0:512],
                                start=(kt == 0), stop=(kt == KT - 1),
                            )
                            nc.tensor.matmul(
                                po[p][:, 512:1024],
                                vha[:, kt, base + 65:base + 193],
                                pt[:, 512:1024],
                                start=(kt == 0), stop=(kt == KT - 1),
                            )
                    for p in range(2):
                        qsl = slice(qt * 512, (qt + 1) * 512)
                        nc.vector.tensor_copy(
                            out=OT[p][0:64, qsl], in_=po[p][0:64, 0:512])
                        nc.vector.tensor_copy(
                            out=OT[p][64:128, qsl], in_=po[p][64:128, 512:1024])
                        nc.vector.tensor_copy(
                            out=sums_stage[64:65, p, qsl],
                            in_=po[p][64:65, 0:512])
                        nc.vector.tensor_copy(
                            out=sums_stage[32:33, p, qsl],
                            in_=po[p][32:33, 512:1024])

                # ---- batched normalization ----
                recin = singles.tile([128, 64], F32, tag="recin")
                for h in range(HPC):
                    p, is_b = h // 2, h % 2
                    row = 32 if is_b else 64
                    nc.sync.dma_start(
                        out=recin[:, h * 16:(h + 1) * 16],
                        in_=sums_stage[row:row + 1, p, :])
                recout = singles.tile([128, 64], F32, tag="recout")
                nc.vector.reciprocal(out=recout, in_=recin)
                for h in range(HPC):
                    nc.sync.dma_start(
                        out=rscr_d[h:h + 1, :],
                        in_=recout[:, h * 16:(h + 1) * 16])
                for p in range(2):
                    rbc = singles.tile([128, S], F32, tag=f"rbc{p}", name=f"rbc{p}")
                    for ab in range(2):
                        src = rscr_d[2 * p + ab:2 * p + ab + 1, :]
                        src_bc = bass.AP(
                            tensor=src.tensor,
                            offset=src.offset,
                            ap=[[0, 64], list(src.ap[-1])],
                        )
                        nc.sync.dma_start(
                            out=rbc[ab * 64:(ab + 1) * 64, :], in_=src_bc)
                    nc.vector.tensor_tensor(
                        out=OT[p], in0=OT[p], in1=rbc,
                        op=mybir.AluOpType.mult)

            # ---- phase O: output projection ----
            with tc.tile_pool(name="oyp", bufs=8, space="PSUM") as oyp, \
                 tc.tile_pool(name="ysb", bufs=8) as ysb:
                for ot in range(8):
                    pys = [oyp.tile([128, 512], F32, tag="py", name="py") for _ in range(QT)]
                    for p in range(2):
                        for n in range(QT):
                            nc.tensor.matmul(
                                pys[n],
                                wo_sb[p][:, ot * 128:(ot + 1) * 128],
                                OT[p][:, n * 512:(n + 1) * 512],
                                start=(p == 0),
                                stop=(p == 1),
                            )
                    for n in range(QT):
                        yt = ysb.tile([128, 512], F32, tag="yt", name="yt")
                        nc.vector.tensor_copy(out=yt, in_=pys[n])
                        nc.sync.dma_start(
                            out=yT_d[ot * 128:(ot + 1) * 128,
                                     n * 512:(n + 1) * 512],
                            in_=yt)
    nc.compile()
    return nc


_NC_CACHE = None


def get_nc():
    global _NC_CACHE
    if _NC_CACHE is None:
        _NC_CACHE = build_nc()
    return _NC_CACHE


def prep_in_maps(q, k, v, mask, Wq, bq, Wk, bk, Wv, bv, Wo, bo):
    q = np.asarray(q, np.float32)
    k = np.asarray(k, np.float32)
    v = np.asarray(v, np.float32)
    mask = np.asarray(mask)
    WqT = np.asarray(Wq, np.float32).T
    WkT = np.asarray(Wk, np.float32).T
    WvT = np.asarray(Wv, np.float32).T
    WoT = np.asarray(Wo, np.float32).T
    bq = np.asarray(bq, np.float32)
    bk = np.asarray(bk, np.float32)
    bv = np.asarray(bv, np.float32)

    xT = {}
    keepT = {}
    for b in range(B):
        xT[b] = (
            np.ascontiguousarray(q[b].T).astype(NP_BF16),
            np.ascontiguousarray(k[b].T).astype(NP_BF16),
            np.ascontiguousarray(v[b].T).astype(NP_BF16),
        )
        keepT[b] = np.ascontiguousarray(
            (~mask[b, 0]).T.astype(np.float32)).astype(NP_BF16)

    in_maps = []
    for c in range(N_CORES):
        b = c // 4
        ho = c % 4
        dsl = slice(ho * 256, ho * 256 + 256)
        xqT, xkT, xvT = xT[b]
        in_maps.append({
            "xqT": xqT,
            "xkT": xkT,
            "xvT": xvT,
            "wqT": np.ascontiguousarray(WqT[:, dsl]).astype(NP_BF16),
            "wkT": np.ascontiguousarray(WkT[:, dsl]).astype(NP_BF16),
            "wvT": np.ascontiguousarray(WvT[:, dsl]).astype(NP_BF16),
            "woT": np.ascontiguousarray(WoT[dsl, :]).astype(NP_BF16),
            "bq2": np.ascontiguousarray(bq[dsl]).reshape(2, 128, 1).astype(np.float32),
            "bk2": np.ascontiguousarray(bk[dsl]).reshape(2, 128, 1).astype(np.float32),
            "bvb": np.ascontiguousarray(
                np.broadcast_to(bv[dsl], (128, 256))).astype(NP_BF16),
            "maskT": keepT[b],
        })
    return in_maps


def gather_output(results, bo):
    bo = np.asarray(bo, np.float32)
    y = np.zeros((B, S, DIM), np.float32)
    for c in range(N_CORES):
        y[c // 4] += np.asarray(results[c]["yT"], np.float32).T
    y += bo[None, None, :]
    return y


def kernel(**inputs):
    nc = get_nc()
    in_maps = prep_in_maps(**{k_: inputs[k_] for k_ in (
        "q", "k", "v", "mask", "Wq", "bq", "Wk", "bk", "Wv", "bv", "Wo", "bo")})
    res = bass_utils.run_bass_kernel_spmd(nc, in_maps, list(range(N_CORES)))
    return gather_output(res.results, inputs["bo"])
